# revision 4
# baseline (speedup 1.0000x reference)
"""COLoRA linear kernel for 8 Trainium2 NeuronCores.

Reference computation (per batch element b with task t = task_ids[b]):

    out[b] = x[b] @ W.T + bias
           + cw      * 2 * (x[b] @ shared_A.T)    @ shared_B.T
           + (1-cw)  * 2 * (x[b] @ expert_A[t].T) @ expert_B[t].T
    cw = sigmoid(collab_w)

The rank-8 adapters fold exactly into the dense weight (associativity):

    W_eff[b] = W + cw*2*(shared_B @ shared_A) + (1-cw)*2*(expert_B[t] @ expert_A[t])
    out[b]   = x[b] @ W_eff[b].T + bias

so the device kernel is a single GEMM per core. Sharding is data-parallel
over batch: core c handles batch element c (B == n_cores == 8). The MoE
routing (task_ids gather) happens on the host at dispatch time.

v2 layout (vs the fp32r baseline at ~139-145us):
  * x and W are stored bf16 (fp32 psum accumulate).  Loads drop from
    20.5 MiB to 10.5 MiB per core, so the PE - not HBM - is the only
    roofline (512 matmuls x ~227ns = 116us issue stream).  bf16 also
    enables the HW fast-weight-load path (disabled for fp32 dtypes).
  * The whole bf16 x (64 KiB/partition) plus W (16 KiB/partition) is
    resident in SBUF: all loads are issued up-front on the sync ring in
    exactly PE consumption order, so there is no tile-pool rotation and
    no just-in-time load the PE can stall on.  The fp32r baseline's
    trace showed DMA busy 99% of the kernel at ~330 GB/s with the PE
    waiting on per-macro loads.
  * Stores go on the scalar ring while the sync ring drains the x
    stream, alternating rings only after the loads are done.
  * Warmup matmuls bridge the HAM clock ramp (PE starts at 1.2 GHz and
    needs ~3.4us of sustained activity to reach 2.4 GHz); the baseline's
    12xN=256 warmup was too short - first un-throttle landed at 11.2us.
"""

import os

import numpy as np

import concourse.bass as bass
import concourse.tile as tile
from concourse import bacc, mybir
from concourse.bass_utils import run_bass_kernel_spmd

try:  # tracing (BASS_TRACE) needs the axon NTFF hook; scrub if unavailable
    from antenv.axon_hooks import get_axon_ntff_profile_hook  # noqa: F401
except ImportError:
    os.environ.pop("BASS_TRACE", None)

N_CORES = 8
S = 4096        # rows per core (sequence length; one batch element per core)
D_IN = 1024
D_OUT = 1024
KC = D_IN // 128   # contraction chunks of 128
S_MACRO = 512      # s rows per macro tile (4 psum u-groups of 128)
N_HALF = 512       # psum free dim (one bank)
SCALING = 2.0      # lora alpha/r = 16/8

MM_DT = mybir.dt.bfloat16
N_WARM = 8         # warmup matmuls (N=512 @ 1.2GHz ~ 3.4us = one HAM window)

_PROGRAM = None
LAST_RESULTS = None  # test harness introspection (exec_time_ns when traced)


def _build_program():
    f32 = mybir.dt.float32
    nc = bacc.Bacc("TRN2", debug=False, num_devices=N_CORES)

    xt_d = nc.dram_tensor("xt", [D_IN, S], MM_DT, kind="ExternalInput").ap()
    wt_d = nc.dram_tensor("wt", [D_IN, D_OUT], MM_DT, kind="ExternalInput").ap()
    bb_d = nc.dram_tensor("bb", [128, D_OUT], f32, kind="ExternalInput").ap()
    out_d = nc.dram_tensor("out", [S, D_OUT], f32, kind="ExternalOutput").ap()

    # contraction dim on partitions, chunked by 128
    xt_v = xt_d.rearrange("(k p) s -> p k s", p=128)      # [128, KC, S]
    wt_v = wt_d.rearrange("(k p) o -> p k o", p=128)      # [128, KC, D_OUT]
    # output rows s = t*S_MACRO + u*128 + p
    out_v = out_d.rearrange(
        "(t u p) o -> t u p o", u=S_MACRO // 128, p=128
    )  # [T, 4, 128, D_OUT]

    NT = S // S_MACRO
    NU = S_MACRO // 128
    NH = D_OUT // N_HALF

    with tile.TileContext(nc) as tc:
        with (
            tc.tile_pool(name="const", bufs=1) as cpool,
            tc.tile_pool(name="outp", bufs=4) as opool,
            tc.tile_pool(name="psum", bufs=8, space="PSUM") as ppool,
        ):
            # PE HAM warmup: dummy matmuls with no DMA deps keep the PE busy
            # from cycle 0 so the activity window latches K=8/8 (2.4 GHz)
            # before/while the first real chunks land.
            warm_w = cpool.tile([128, 128], MM_DT)
            warm_x = cpool.tile([128, N_HALF], MM_DT)
            nc.vector.memset(warm_w[:], 0.0)
            nc.vector.memset(warm_x[:], 0.0)
            warm_ps = ppool.tile([128, N_HALF], f32, tag="ps")
            for _ in range(N_WARM):
                nc.tensor.matmul(
                    warm_ps[:], warm_w[:], warm_x[:], start=True, stop=True
                )

            # weights per k-chunk on the ACT ring so W[k0] lands ~1us in
            wtile = cpool.tile([128, KC, D_OUT], MM_DT)
            for k in range(KC):
                nc.scalar.dma_start(wtile[:, k, :], wt_v[:, k, :])
            btile = cpool.tile([128, D_OUT], f32)
            nc.scalar.dma_start(btile[:], bb_d[:])

            # whole x resident in SBUF (64 KiB/partition bf16), loaded on the
            # sync ring in PE consumption order: macro 0 split per k-chunk
            # (first matmul gates on a 128 KiB DMA, not a 1 MiB one), then
            # one DMA per macro.
            xfull = cpool.tile([128, KC, S], MM_DT)
            for k in range(KC):
                nc.sync.dma_start(xfull[:, k, :S_MACRO], xt_v[:, k, :S_MACRO])
            for t in range(1, NT):
                s_sl = slice(t * S_MACRO, (t + 1) * S_MACRO)
                nc.sync.dma_start(xfull[:, :, s_sl], xt_v[:, :, s_sl])

            # macro 0 ramp: k outermost with all 8 psum groups open - each
            # arriving (x[k], W[k]) chunk pair feeds 8 matmuls, so the PE
            # stays continuously busy through the HAM ramp while the rest of
            # the x stream lands.
            otiles, pss = [], []
            for u in range(NU):
                otile = opool.tile([128, D_OUT], f32)
                otiles.append(otile)
                for _h in range(NH):
                    ps = ppool.tile([128, N_HALF], f32, tag="ps")
                    pss.append(ps)
            for k in range(KC):
                for u in range(NU):
                    for h in range(NH):
                        nc.tensor.matmul(
                            pss[u * NH + h][:],
                            xfull[:, k, u * 128 : (u + 1) * 128],
                            wtile[:, k, h * N_HALF : (h + 1) * N_HALF],
                            start=(k == 0),
                            stop=(k == KC - 1),
                        )
            for u in range(NU):
                for h in range(NH):
                    nc.vector.tensor_add(
                        otiles[u][:, h * N_HALF : (h + 1) * N_HALF],
                        pss[u * NH + h][:],
                        btile[:, h * N_HALF : (h + 1) * N_HALF],
                    )
                nc.scalar.dma_start(out_v[0, u], otiles[u][:])

            # steady state: u-outer, k-inner; 2 psum banks per u-tile, 4
            # u-tiles in flight across the 8-bank pool.
            for t in range(1, NT):
                for u in range(NU):
                    otile = opool.tile([128, D_OUT], f32)
                    pss = []
                    for _h in range(NH):
                        ps = ppool.tile([128, N_HALF], f32, tag="ps")
                        pss.append(ps)
                    for k in range(KC):
                        # both output halves per k share the stationary lhsT
                        for h in range(NH):
                            nc.tensor.matmul(
                                pss[h][:],
                                xfull[:, k, t * S_MACRO + u * 128 : t * S_MACRO + (u + 1) * 128],
                                wtile[:, k, h * N_HALF : (h + 1) * N_HALF],
                                start=(k == 0),
                                stop=(k == KC - 1),
                            )
                    for h in range(NH):
                        # evacuate psum with fused bias add
                        nc.vector.tensor_add(
                            otile[:, h * N_HALF : (h + 1) * N_HALF],
                            pss[h][:],
                            btile[:, h * N_HALF : (h + 1) * N_HALF],
                        )
                    if t == NT - 1 and u == NU - 1:
                        # final tile: store halves on both rings as soon as
                        # each bias-add lands - halves the last flush the
                        # exit drain waits on
                        for h in range(NH):
                            eng = nc.scalar if h == 0 else nc.sync
                            eng.dma_start(
                                out_v[t, u][:, h * N_HALF : (h + 1) * N_HALF],
                                otile[:, h * N_HALF : (h + 1) * N_HALF],
                            )
                    elif t < 4:
                        # sync ring still owns the x-load stream; keep all
                        # early stores on the scalar ring
                        nc.scalar.dma_start(out_v[t, u], otile[:])
                    else:
                        store_eng = nc.scalar if (t * NU + u) % 2 == 0 else nc.sync
                        store_eng.dma_start(out_v[t, u], otile[:])

    nc.compile()
    return nc


def _get_program():
    global _PROGRAM
    if _PROGRAM is None:
        _PROGRAM = _build_program()
    return _PROGRAM


def kernel(x, task_ids, W, b, shared_A, shared_B, expert_A, expert_B, collab_w):
    global LAST_RESULTS
    x = np.asarray(x, dtype=np.float32)
    task_ids = np.asarray(task_ids)
    W = np.asarray(W, dtype=np.float32)
    b = np.asarray(b, dtype=np.float32)
    B = x.shape[0]
    assert B == N_CORES and x.shape[1:] == (S, D_IN)

    cw = np.float32(1.0 / (1.0 + np.exp(-np.float64(collab_w))))
    w_shared = (
        W
        + np.float32(cw * SCALING)
        * (np.asarray(shared_B, np.float32) @ np.asarray(shared_A, np.float32))
    ).astype(np.float32)
    ce = np.float32((1.0 - cw) * SCALING)

    np_in = mybir.dt.np(MM_DT)
    bb = np.ascontiguousarray(np.broadcast_to(b, (128, D_OUT)), dtype=np.float32)
    in_maps = []
    for bi in range(B):
        t = int(task_ids[bi])
        w_eff = w_shared + ce * (
            np.asarray(expert_B[t], np.float32) @ np.asarray(expert_A[t], np.float32)
        )
        in_maps.append(
            {
                "xt": np.ascontiguousarray(x[bi].T).astype(np_in),
                "wt": np.ascontiguousarray(w_eff.T).astype(np_in),
                "bb": bb,
            }
        )

    nc = _get_program()
    LAST_RESULTS = run_bass_kernel_spmd(nc, in_maps, list(range(N_CORES)))
    out = np.stack(
        [LAST_RESULTS.results[c]["out"] for c in range(N_CORES)], axis=0
    )
    return np.ascontiguousarray(out, dtype=np.float32)


# revision 6
# speedup vs baseline: 1.0463x; 1.0463x over previous
"""COLoRA linear kernel for 8 Trainium2 NeuronCores.

Reference computation (per batch element b with task t = task_ids[b]):

    out[b] = x[b] @ W.T + bias
           + cw      * 2 * (x[b] @ shared_A.T)    @ shared_B.T
           + (1-cw)  * 2 * (x[b] @ expert_A[t].T) @ expert_B[t].T
    cw = sigmoid(collab_w)

The rank-8 adapters fold exactly into the dense weight (associativity):

    W_eff[b] = W + cw*2*(shared_B @ shared_A) + (1-cw)*2*(expert_B[t] @ expert_A[t])
    out[b]   = x[b] @ W_eff[b].T + bias

so the device kernel is a single GEMM per core; core c handles batch
element c (B == n_cores == 8); task_ids routing happens on the host.

v3 notes (traced on HW):
  * Matmuls stay float32r: measured steady spacing is 227 ns/MM; a bf16
    variant measured 259 ns/MM (likely the FWL weight-load path), a
    +16 us regression that swamps any DMA win.
  * Outputs are stored bf16 (host upcasts): stores drop 16.8->8.4 MB,
    keeping total HBM traffic (~29 MB) well under the PE stream time,
    so loads never pace the PE after the ramp.
  * Whole x + W are SBUF-resident (128 KiB + 32 KiB per partition),
    loads issued up-front in consumption order; no pool rotation.
  * Two-phase ramp: phase A does the h0 output half of rows 0..1023
    k-outer across 8 psum banks (paced by the interleaved x/W-h0
    arrivals); phase B does the h1 half u-outer from SBUF.  This halves
    the front-load W demand so the 4 MiB fp32 W never stalls the ramp.
  * Warmup matmuls are kept live (fed via a zero-add into the bias
    tile): a previous version's warmups into a never-read psum were
    dead-code-eliminated and the PE sat cold until 12.7 us.
"""

import os

import numpy as np

import concourse.bass as bass
import concourse.tile as tile
from concourse import bacc, mybir
from concourse.bass_utils import run_bass_kernel_spmd

try:  # tracing (BASS_TRACE) needs the axon NTFF hook; scrub if unavailable
    from antenv.axon_hooks import get_axon_ntff_profile_hook  # noqa: F401
except ImportError:
    os.environ.pop("BASS_TRACE", None)

N_CORES = 8
S = 4096        # rows per core (sequence length; one batch element per core)
D_IN = 1024
D_OUT = 1024
KC = D_IN // 128   # contraction chunks of 128
S_MACRO = 512      # s rows per steady macro tile
N_HALF = 512       # psum free dim (one bank)
SCALING = 2.0      # lora alpha/r = 16/8

MM_DT = mybir.dt.float32r
OUT_DT = mybir.dt.bfloat16
N_WARM = 4

_PROGRAM = None
LAST_RESULTS = None  # test harness introspection (exec_time_ns when traced)


def _build_program():
    f32 = mybir.dt.float32
    nc = bacc.Bacc("TRN2", debug=False, num_devices=N_CORES)

    xt_d = nc.dram_tensor("xt", [D_IN, S], MM_DT, kind="ExternalInput").ap()
    wt_d = nc.dram_tensor("wt", [D_IN, D_OUT], MM_DT, kind="ExternalInput").ap()
    bb_d = nc.dram_tensor("bb", [128, D_OUT], OUT_DT, kind="ExternalInput").ap()
    out_d = nc.dram_tensor("out", [S, D_OUT], OUT_DT, kind="ExternalOutput").ap()

    # contraction dim on partitions, chunked by 128
    xt_v = xt_d.rearrange("(k p) s -> p k s", p=128)      # [128, KC, S]
    wt_v = wt_d.rearrange("(k p) o -> p k o", p=128)      # [128, KC, D_OUT]
    out_v = out_d.rearrange("(n p) o -> n p o", p=128)    # [32, 128, D_OUT]

    NT = S // S_MACRO
    NU = S_MACRO // 128
    NH = D_OUT // N_HALF
    NG = 8  # ramp covers rows 0..NG*128 (macros t=0,1)

    with tile.TileContext(nc) as tc:
        with (
            tc.tile_pool(name="const", bufs=1) as cpool,
            tc.tile_pool(name="outp", bufs=4) as opool,
            tc.tile_pool(name="psum", bufs=8, space="PSUM") as ppool,
        ):
            # PE HAM warmup: one live accumulation group with no DMA deps.
            # The result (exact zeros) is grafted into the bias tile below so
            # dead-code elimination cannot drop it.
            warm_w = cpool.tile([128, 128], mybir.dt.bfloat16)
            warm_x = cpool.tile([128, N_HALF], mybir.dt.bfloat16)
            nc.gpsimd.memset(warm_w[:], 0.0)
            nc.gpsimd.memset(warm_x[:], 0.0)
            warm_ps = ppool.tile([128, N_HALF], f32, tag="ps")
            for i in range(N_WARM):
                nc.tensor.matmul(
                    warm_ps[:], warm_w[:], warm_x[:],
                    start=(i == 0), stop=(i == N_WARM - 1),
                )
            # evacuate immediately (no DMA dep) so the warm psum bank frees
            # before the ramp needs all 8 banks
            warm_sb = cpool.tile([128, N_HALF], f32)
            nc.vector.tensor_scalar_add(warm_sb[:], warm_ps[:], 0.0)

            # W h0 chunks first (the ramp's only W need), then bias, then h1
            wtile = cpool.tile([128, KC, D_OUT], MM_DT)
            for k in range(KC):
                nc.scalar.dma_start(wtile[:, k, :N_HALF], wt_v[:, k, :N_HALF])
            btile = cpool.tile([128, D_OUT], OUT_DT)
            nc.scalar.dma_start(btile[:], bb_d[:])
            for k in range(KC):
                nc.scalar.dma_start(wtile[:, k, N_HALF:], wt_v[:, k, N_HALF:])

            # x: rows 0..1023 per (k, t) in ramp consumption order, then one
            # DMA per remaining macro
            xfull = cpool.tile([128, KC, S], MM_DT)
            for k in range(KC):
                for t in range(2):
                    s_sl = slice(t * S_MACRO, (t + 1) * S_MACRO)
                    nc.sync.dma_start(xfull[:, k, s_sl], xt_v[:, k, s_sl])
            for t in range(2, NT):
                s_sl = slice(t * S_MACRO, (t + 1) * S_MACRO)
                nc.sync.dma_start(xfull[:, :, s_sl], xt_v[:, :, s_sl])

            # graft the (zero) warmup result into the bias tile: keeps the
            # warmup live and costs one DVE op before the first evacuation
            nc.vector.tensor_add(btile[:, :N_HALF], btile[:, :N_HALF], warm_sb[:])

            # phase A ramp: h0 half of rows 0..1023, k outermost across all
            # 8 psum banks - each arriving (x[t0,k], x[t1,k], Wh0[k]) triple
            # feeds 8 matmuls so the PE stays busy through the HAM ramp.
            psA, otA = [], []
            for g in range(NG):
                ps = ppool.tile([128, N_HALF], f32, tag="ps")
                psA.append(ps)
                ot = opool.tile([128, N_HALF], OUT_DT)
                otA.append(ot)
            for k in range(KC):
                for g in range(NG):
                    nc.tensor.matmul(
                        psA[g][:],
                        xfull[:, k, g * 128 : (g + 1) * 128],
                        wtile[:, k, :N_HALF],
                        start=(k == 0),
                        stop=(k == KC - 1),
                    )
            for g in range(NG):
                nc.vector.tensor_add(otA[g][:], psA[g][:], btile[:, :N_HALF])
                nc.scalar.dma_start(out_v[g][:, :N_HALF], otA[g][:])

            # phase B: h1 half of rows 0..1023, u-outer (all data resident;
            # consumes banks as phase A's staggered evacuations free them)
            for g in range(NG):
                ps = ppool.tile([128, N_HALF], f32, tag="ps")
                ot = opool.tile([128, N_HALF], OUT_DT)
                for k in range(KC):
                    nc.tensor.matmul(
                        ps[:],
                        xfull[:, k, g * 128 : (g + 1) * 128],
                        wtile[:, k, N_HALF:],
                        start=(k == 0),
                        stop=(k == KC - 1),
                    )
                nc.vector.tensor_add(ot[:], ps[:], btile[:, N_HALF:])
                nc.scalar.dma_start(out_v[g][:, N_HALF:], ot[:])

            # steady state: macros t=2..7, u-outer k-inner, 2 banks per
            # u-tile, 4 u-tiles in flight across the 8-bank pool
            for t in range(2, NT):
                for u in range(NU):
                    n = t * NU + u
                    if t == NT - 1 and u == NU - 1:
                        # final tile: finish h0 entirely first so its
                        # bias-add + store overlap h1's matmuls, then fire
                        # the halves on separate rings
                        for h in range(NH):
                            ps = ppool.tile([128, N_HALF], f32, tag="ps")
                            ot = opool.tile([128, N_HALF], OUT_DT)
                            for k in range(KC):
                                nc.tensor.matmul(
                                    ps[:],
                                    xfull[:, k, n * 128 : (n + 1) * 128],
                                    wtile[:, k, h * N_HALF : (h + 1) * N_HALF],
                                    start=(k == 0),
                                    stop=(k == KC - 1),
                                )
                            nc.vector.tensor_add(
                                ot[:], ps[:],
                                btile[:, h * N_HALF : (h + 1) * N_HALF],
                            )
                            eng = nc.scalar if h == 0 else nc.sync
                            eng.dma_start(
                                out_v[n][:, h * N_HALF : (h + 1) * N_HALF],
                                ot[:],
                            )
                        continue
                    otile = opool.tile([128, D_OUT], OUT_DT)
                    pss = []
                    for _h in range(NH):
                        ps = ppool.tile([128, N_HALF], f32, tag="ps")
                        pss.append(ps)
                    for k in range(KC):
                        # both output halves per k share the stationary lhsT
                        for h in range(NH):
                            nc.tensor.matmul(
                                pss[h][:],
                                xfull[:, k, n * 128 : (n + 1) * 128],
                                wtile[:, k, h * N_HALF : (h + 1) * N_HALF],
                                start=(k == 0),
                                stop=(k == KC - 1),
                            )
                    for h in range(NH):
                        nc.vector.tensor_add(
                            otile[:, h * N_HALF : (h + 1) * N_HALF],
                            pss[h][:],
                            btile[:, h * N_HALF : (h + 1) * N_HALF],
                        )
                    if t < 4:
                        # sync ring still owns the x-load stream
                        nc.scalar.dma_start(out_v[n], otile[:])
                    else:
                        store_eng = nc.scalar if n % 2 == 0 else nc.sync
                        store_eng.dma_start(out_v[n], otile[:])

    nc.compile()
    return nc


def _get_program():
    global _PROGRAM
    if _PROGRAM is None:
        _PROGRAM = _build_program()
    return _PROGRAM


def kernel(x, task_ids, W, b, shared_A, shared_B, expert_A, expert_B, collab_w):
    global LAST_RESULTS
    x = np.asarray(x, dtype=np.float32)
    task_ids = np.asarray(task_ids)
    W = np.asarray(W, dtype=np.float32)
    b = np.asarray(b, dtype=np.float32)
    B = x.shape[0]
    assert B == N_CORES and x.shape[1:] == (S, D_IN)

    cw = np.float32(1.0 / (1.0 + np.exp(-np.float64(collab_w))))
    w_shared = (
        W
        + np.float32(cw * SCALING)
        * (np.asarray(shared_B, np.float32) @ np.asarray(shared_A, np.float32))
    ).astype(np.float32)
    ce = np.float32((1.0 - cw) * SCALING)

    np_in = mybir.dt.np(MM_DT)
    np_out = mybir.dt.np(OUT_DT)
    bb = np.ascontiguousarray(np.broadcast_to(b, (128, D_OUT))).astype(np_out)
    in_maps = []
    for bi in range(B):
        t = int(task_ids[bi])
        w_eff = w_shared + ce * (
            np.asarray(expert_B[t], np.float32) @ np.asarray(expert_A[t], np.float32)
        )
        in_maps.append(
            {
                "xt": np.ascontiguousarray(x[bi].T).astype(np_in),
                "wt": np.ascontiguousarray(w_eff.T).astype(np_in),
                "bb": bb,
            }
        )

    nc = _get_program()
    LAST_RESULTS = run_bass_kernel_spmd(nc, in_maps, list(range(N_CORES)))
    out = np.stack(
        [np.asarray(LAST_RESULTS.results[c]["out"]) for c in range(N_CORES)],
        axis=0,
    )
    return np.ascontiguousarray(out.astype(np.float32))
